# revision 1
# baseline (speedup 1.0000x reference)
"""Trainium2 Bass kernel for the detection-loss problem.

Strategy: data-parallel over the batch dim — each of the 8 NeuronCores
processes one image end-to-end and emits per-partition partial sums; the
host combines them into the final scalar losses.

Per-core pipeline (all f32):
  stage A  decode anchors image-wide ([128, 720] ops)
  pair     [128, (16, 32)] tiles: IoU vs 32 boxes via min/max/sub chain,
           reciprocal_approx_fast, reduce_max -> per-anchor max IoU,
           argmax one-hot mask -> DVE 32x32 block-transpose -> PE matmuls
           gather the matched box payload (centers, log-sizes, label)
  CE       logsumexp over 21 classes + matched-label logit via iota-eq
  reg      affine-encode targets from gathered payload, smooth-L1
  sums     scalar_tensor_tensor accum_out -> staging [128, 45*8] -> DRAM

Anchor indexing: n = t*2048 + p*16 + f  (t tile, p partition, f free),
padded N 90000 -> 92160 with valid=0 rows.
"""

import numpy as np

B, A, C, H, W = 8, 9, 21, 100, 100
M = 32
N = A * H * W            # 90000
F = 24                   # anchor groups per tile along free dim
TPB = 128 * F            # anchors per tile = 2048
NT = 30                  # tiles per image
NPAD = NT * TPB          # 92160
NQ = 8                   # staging quantities per tile
POS_THR, NEG_THR = 0.25, 0.1
# thresholds in ln-q space: iou >= t  <=>  q >= t/(1+t)
LN_POS = float(np.log(0.25 / 1.25))
LN_NEG = float(np.log(0.1 / 1.1))

_CACHE = {}
LAST_RESULTS = None


def _to_tiled(arr_nk):
    """[NPAD, k] -> [128, NT*F*k] p-major host tiling (f,k innermost)."""
    k = arr_nk.shape[1]
    return (
        arr_nk.reshape(NT, 128, F * k).transpose(1, 0, 2).reshape(128, NT * F * k)
    ).astype(np.float32).copy()


def _pad_rows(x, pad_val=0.0):
    out = np.full((NPAD, x.shape[1]), pad_val, np.float32)
    out[:N] = x
    return out


def _host_prep_shared(anchors):
    anchors = np.asarray(anchors, np.float32)
    aw = anchors[:, 2] - anchors[:, 0]
    ah = anchors[:, 3] - anchors[:, 1]
    acx = anchors[:, 0] + 0.5 * aw
    acy = anchors[:, 1] + 0.5 * ah

    def padded(col, pv):
        full = np.full(NPAD, pv, np.float32)
        full[:N] = col
        return full

    c1x = padded(aw / 2, 1.0)
    c2x = padded(acx - aw / 4, -4000.0)
    c1y = padded(ah / 2, 1.0)
    c2y = padded(acy - ah / 4, -4000.0)
    ones = np.ones(N, np.float32)
    s1 = padded(2.0 / aw, 1.0)
    s2 = padded(2.0 / ah, 1.0)
    b1 = padded(0.5 - 2.0 * acx / aw, 0.0)
    b2 = padded(0.5 - 2.0 * acy / ah, 0.0)
    b3 = padded(-np.log(aw), 0.0)
    b4 = padded(-np.log(ah), 0.0)
    valid = padded(ones, 0.0)

    aca = _to_tiled(np.stack([c1x, c2x, c1y, c2y], 1))
    acr = _to_tiled(np.stack([s1, s2, padded(ones, 1.0), padded(ones, 1.0),
                              b1, b2, b3, b4], 1))
    avv = _to_tiled(valid[:, None])
    return aca, acr, avv


def _host_prep_image(cls_i, reg_i, tb_i, tl_i):
    f32 = np.float32
    cls_flat = np.transpose(np.asarray(cls_i, f32), (0, 2, 3, 1)).reshape(N, C)
    reg_flat = np.transpose(np.asarray(reg_i, f32), (0, 2, 3, 1)).reshape(N, 4)
    clsb = _to_tiled(_pad_rows(cls_flat))
    regb = _to_tiled(_pad_rows(reg_flat))

    tb = np.asarray(tb_i, f32)
    tl = np.asarray(tl_i).astype(f32)
    bw = tb[:, 2] - tb[:, 0]
    bh = tb[:, 3] - tb[:, 1]
    bcx = tb[:, 0] + 0.5 * bw
    bcy = tb[:, 1] + 0.5 * bh
    areaB = bw * bh
    boxr = np.tile(
        np.concatenate([tb[:, 0], tb[:, 2], tb[:, 1], tb[:, 3], areaB]).astype(f32),
        (128, 1),
    ).copy()  # [128, 160]: bx0 | bx1 | by0 | by1 | areaB
    p32 = np.stack([bcx, bcy, np.log(bw), np.log(bh), tl,
                    np.zeros_like(bw), np.zeros_like(bw), np.zeros_like(bw)], 1)
    ptab = np.tile(p32.astype(f32), (4, 1)).copy()  # [128, 8]
    return clsb, regb, boxr, ptab


def _build_nc(repeat=1):
    import concourse.bacc as bacc
    import concourse.mybir as mybir
    from concourse.tile import TileContext

    dt = mybir.dt
    f32 = dt.float32
    op = mybir.AluOpType
    act = mybir.ActivationFunctionType
    X = mybir.AxisListType.X

    nc = bacc.Bacc("TRN2", target_bir_lowering=False, debug=False, num_devices=8)

    clsb_d = nc.dram_tensor("clsb", [128, NT * F * C], f32, kind="ExternalInput")
    regb_d = nc.dram_tensor("regb", [128, NT * F * 4], f32, kind="ExternalInput")
    aca_d = nc.dram_tensor("aca", [128, NT * F * 4], f32, kind="ExternalInput")
    acr_d = nc.dram_tensor("acr", [128, NT * F * 8], f32, kind="ExternalInput")
    avv_d = nc.dram_tensor("avv", [128, NT * F], f32, kind="ExternalInput")
    boxr_d = nc.dram_tensor("boxr", [128, 5 * M], f32, kind="ExternalInput")
    ptab_d = nc.dram_tensor("ptab", [128, 8], f32, kind="ExternalInput")
    stage_d = nc.dram_tensor("stage", [128, NT * NQ], f32, kind="ExternalOutput")
    gmat_d = nc.dram_tensor("gmat", [M, C], f32, kind="ExternalOutput")

    NTF = NT * F  # 720

    with TileContext(nc) as tc:
        with (
            tc.tile_pool(name="const", bufs=1) as constp,
            tc.tile_pool(name="anc", bufs=1) as ancp,
            tc.tile_pool(name="clsp", bufs=4) as clsp,
            tc.tile_pool(name="pair", bufs=2) as pairp,
            tc.tile_pool(name="regs", bufs=2) as regsp,
            tc.tile_pool(name="small", bufs=3) as smallp,
            tc.tile_pool(name="ps", bufs=2, space="PSUM") as psp,
            tc.tile_pool(name="psg", bufs=1, space="PSUM") as psgp,
        ):
            # ---- load constants / inputs ----
            boxr = constp.tile([128, 5 * M], f32, tag="boxr", name="boxr")
            nc.sync.dma_start(boxr[:], boxr_d[:])
            ptab = constp.tile([128, 8], f32, tag="ptab", name="ptab")
            nc.sync.dma_start(ptab[:], ptab_d[:])
            regall = constp.tile([128, NTF * 4], f32, tag="regall", name="regall")
            nc.sync.dma_start(regall[:], regb_d[:])

            acrall = constp.tile([128, NTF * 8], f32, tag="acrall", name="acrall")
            nc.sync.dma_start(acrall[:], acr_d[:])
            avvall = constp.tile([128, NTF], f32, tag="avvall", name="avvall")
            nc.sync.dma_start(avvall[:], avv_d[:])

            staging = constp.tile([128, NT * NQ], f32, tag="staging", name="staging")
            gmat_ps = psgp.tile([M, C], f32, tag="gmat_ps", name="gmat_ps")
            nc.vector.memset(staging[:], 0.0)

            rview = regall[:].rearrange("p (n k) -> p n k", k=4)
            sview = acrall[:].rearrange("p (n k) -> p n k", k=8)

            # ---- stage A: decode (image-wide) ----
            with tc.tile_pool(name="tmpa", bufs=1) as tmpap:
                acaall = tmpap.tile([128, NTF * 4], f32, tag="acaall", name="acaall")
                nc.sync.dma_start(acaall[:], aca_d[:])
                aview = acaall[:].rearrange("p (n k) -> p n k", k=4)

                def anc_tile(tag):
                    return ancp.tile([128, NTF], f32, tag=tag, name=tag)

                def tmp_tile(tag):
                    return tmpap.tile([128, NTF], f32, tag=tag, name=tag)

                ewh = tmpap.tile([128, NTF * 2], f32, tag="ewh", name="ewh")
                ewh_v = ewh[:].rearrange("p (n k) -> p n k", k=2)
                nc.scalar.activation(ewh_v, rview[:, :, 2:4], act.Exp)

                hx = anc_tile("hx")
                nc.vector.tensor_tensor(hx[:], ewh_v[:, :, 0], aview[:, :, 0], op.mult)
                hy = anc_tile("hy")
                nc.vector.tensor_tensor(hy[:], ewh_v[:, :, 1], aview[:, :, 2], op.mult)
                tcx = tmp_tile("tcx")
                nc.vector.tensor_tensor(tcx[:], rview[:, :, 0], aview[:, :, 0], op.mult)
                cx = tmp_tile("cx")
                nc.vector.tensor_tensor(cx[:], tcx[:], aview[:, :, 1], op.add)
                tcy = tmp_tile("tcy")
                nc.vector.tensor_tensor(tcy[:], rview[:, :, 1], aview[:, :, 2], op.mult)
                cy = tmp_tile("cy")
                nc.vector.tensor_tensor(cy[:], tcy[:], aview[:, :, 3], op.add)
                dx0 = anc_tile("dx0")
                nc.vector.tensor_tensor(dx0[:], cx[:], hx[:], op.subtract)
                dx1 = anc_tile("dx1")
                nc.vector.tensor_tensor(dx1[:], cx[:], hx[:], op.add)
                dy0 = anc_tile("dy0")
                nc.vector.tensor_tensor(dy0[:], cy[:], hy[:], op.subtract)
                dy1 = anc_tile("dy1")
                nc.vector.tensor_tensor(dy1[:], cy[:], hy[:], op.add)
                areaq = anc_tile("areaq")  # hx*hy = areaA/4
                nc.vector.tensor_tensor(areaq[:], hx[:], hy[:], op.mult)

            PFM = [128, F, M]

            def bcast_anchor(tile_all, t):
                return tile_all[:, t * F:(t + 1) * F].unsqueeze(2).broadcast_to(PFM)

            def bcast_box(col0):
                return boxr[:, col0 * M:(col0 + 1) * M].unsqueeze(1).broadcast_to(PFM)

            # ---- per-tile main loop (optionally HW-repeated for timing) ----
            import contextlib
            rep_ctx = tc.For_i(0, repeat, 1) if repeat > 1 else contextlib.nullcontext()
            with rep_ctx:
              for t in range(NT):
                  tf = slice(t * F, (t + 1) * F)

                  def ptile(tag, bufs=None):
                      return pairp.tile([128, F * M], f32, tag=tag, name=tag, bufs=bufs)

                  tx1 = ptile("tx1", bufs=3)
                  tx1v = tx1[:].rearrange("p (f m) -> p f m", m=M)
                  nc.vector.tensor_tensor(tx1v, bcast_anchor(dx1, t), bcast_box(1), op.min)
                  nwx = ptile("nwx", bufs=3)
                  nwxv = nwx[:].rearrange("p (f m) -> p f m", m=M)
                  nc.vector.tensor_tensor(nwxv, bcast_anchor(dx0, t), bcast_box(0), op.max)
                  nc.vector.tensor_tensor(nwxv, nwxv, tx1v, op.subtract)
                  ty1 = ptile("ty1")
                  ty1v = ty1[:].rearrange("p (f m) -> p f m", m=M)
                  nc.vector.tensor_tensor(ty1v, bcast_anchor(dy1, t), bcast_box(3), op.min)
                  nwy = ptile("nwy", bufs=3)
                  nwyv = nwy[:].rearrange("p (f m) -> p f m", m=M)
                  nc.vector.tensor_tensor(nwyv, bcast_anchor(dy0, t), bcast_box(2), op.max)
                  nc.vector.tensor_tensor(nwyv, nwyv, ty1v, op.subtract)

                  ir = ptile("ir", bufs=3)
                  irv = ir[:].rearrange("p (f m) -> p f m", m=M)
                  # ir = min(nwx,0)*nwy ; = inter > 0 for overlapping pairs, <=0 else
                  nc.vector.scalar_tensor_tensor(irv, nwxv, 0.0, nwyv, op.min, op.mult)
                  # iou = q/(1-q) with q = inter/(areaA+areaB): monotone in q, so
                  # argmax/thresholds can use ln q = Ln(inter) - Ln(areaA+areaB).
                  sab = ptile("sab", bufs=3)
                  sabv = sab[:].rearrange("p (f m) -> p f m", m=M)
                  nc.vector.scalar_tensor_tensor(sabv, bcast_anchor(areaq, t), 4.0,
                                                 bcast_box(4), op.mult, op.add)
                  # clamp to a positive floor so HW Ln never sees <= 0
                  # (zero-overlap pairs become lq = ln(1e-30/sab) ~ -69, never max)
                  nc.vector.tensor_scalar(ir[:], ir[:], 1e-30, None, op.max)
                  nc.scalar.activation(ir[:], ir[:], act.Ln)
                  nc.scalar.activation(sab[:], sab[:], act.Ln)
                  lq = ir
                  lqv = irv
                  nc.vector.tensor_tensor(lq[:], lq[:], sab[:], op.subtract)

                  mx = smallp.tile([128, F], f32, tag="mx", name="mx")
                  nc.vector.tensor_reduce(mx[:], lqv, axis=X, op=op.max)

                  # posf/negf first (from mx), then fold pos into the argmax
                  # mask: mxp = mx + 1e30*(1-posf) selects nothing on neg rows.
                  posf = smallp.tile([128, F], f32, tag="posf", name="posf")
                  negf = smallp.tile([128, F], f32, tag="negf", name="negf")
                  np_acc = staging[:, t * NQ + 0: t * NQ + 1]
                  nn_acc = staging[:, t * NQ + 1: t * NQ + 2]
                  nc.vector.scalar_tensor_tensor(
                      posf[:], mx[:], LN_POS, avvall[:, tf], op.is_ge, op.mult,
                      accum_out=np_acc)
                  nc.vector.scalar_tensor_tensor(
                      negf[:], mx[:], LN_NEG, avvall[:, tf], op.is_ge, op.subtract,
                      accum_out=nn_acc)
                  bigu = smallp.tile([128, F], f32, tag="bigu", name="bigu")
                  nc.vector.tensor_scalar(bigu[:], posf[:], -1e30, 1e30, op.mult, op.add)
                  mxp = smallp.tile([128, F], f32, tag="mxp", name="mxp")
                  # clamp -inf (zero-overlap rows) so -1e30 + 1e30 = 0 > all lq
                  nc.vector.scalar_tensor_tensor(mxp[:], mx[:], -1e30, bigu[:],
                                                 op.max, op.add)

                  amask = ptile("amask", bufs=3)
                  amv = amask[:].rearrange("p (f m) -> p f m", m=M)
                  nc.vector.tensor_tensor(
                      amv, lqv, mxp[:].unsqueeze(2).broadcast_to(PFM), op.is_ge)
                  tmask = ptile("tmask", bufs=3)
                  nc.vector.transpose(tmask[:], amask[:])
                  tmv = tmask[:].rearrange("p (f m) -> p f m", m=M)

                  # payload gather: per (pb, f) 32x32 block matmul
                  g = psp.tile([128, F * NQ], f32, tag="g", name="g")
                  gv = g[:].rearrange("p (f k) -> p f k", k=NQ)
                  import os as _os
                  if _os.environ.get("SKIP_PE", "0") != "1":
                      for pb in range(4):
                          rows = slice(pb * 32, (pb + 1) * 32)
                          for f in range(F):
                              nc.tensor.matmul(
                                  gv[rows, f, :], tmv[rows, f, :], ptab[rows, :],
                                  start=True, stop=True,
                                  tile_position=(pb * 32, pb * 32))

                  # ---- CE ----
                  ct = clsp.tile([128, F * C], f32, tag="ct", name="ct")
                  nc.sync.dma_start(ct[:], clsb_d[:, t * F * C:(t + 1) * F * C])
                  ctv = ct[:].rearrange("p (f c) -> p f c", c=C)
                  et = clsp.tile([128, F * C], f32, tag="et", name="et")
                  nc.scalar.activation(et[:], ct[:], act.Exp)
                  etv = et[:].rearrange("p (f c) -> p f c", c=C)
                  se = smallp.tile([128, F], f32, tag="se", name="se")
                  nc.vector.tensor_reduce(se[:], etv, axis=X, op=op.add)
                  lse = smallp.tile([128, F], f32, tag="lse", name="lse")
                  nc.scalar.activation(lse[:], se[:], act.Ln)
                  cls0 = smallp.tile([128, F], f32, tag="cls0", name="cls0")
                  nc.gpsimd.tensor_copy(cls0[:], ctv[:, :, 0])

                  # G[m, c] += sum_n Apos[n, m] * cls[n, c]  (PE, contraction
                  # over anchors); host finishes S2 = sum_m G[m, label_m].
                  amv2 = amask[:].rearrange("p (f m) -> p f m", m=M)
                  if _os.environ.get("SKIP_PE", "0") != "1":
                      for f in range(F):
                          nc.tensor.matmul(
                              gmat_ps[:, :], amv2[:, f, :], ctv[:, f, :],
                              start=(t == 0 and f == 0),
                              stop=(t == NT - 1 and f == F - 1),
                              tile_position=(0, 0))

                  # ---- reg ----
                  def rtile(tag):
                      return regsp.tile([128, F * 4], f32, tag=tag, name=tag)

                  u = rtile("u")
                  uv = u[:].rearrange("p (f k) -> p f k", k=4)
                  nc.vector.tensor_tensor(uv, gv[:, :, 0:4], sview[:, tf, 0:4], op.mult)
                  nc.vector.tensor_tensor(uv, uv, sview[:, tf, 4:8], op.add)
                  dd = rtile("dd")
                  ddv = dd[:].rearrange("p (f k) -> p f k", k=4)
                  nc.vector.tensor_tensor(ddv, rview[:, tf, :], uv, op.subtract)
                  ad = rtile("ad")
                  nc.scalar.activation(ad[:], dd[:], act.Abs)
                  cc = rtile("cc")
                  nc.gpsimd.tensor_scalar(cc[:], ad[:], 1.0, None, op.min)
                  q1 = rtile("q1")
                  nc.vector.tensor_tensor(q1[:], ad[:], cc[:], op.mult)
                  nc.vector.scalar_tensor_tensor(cc[:], cc[:], -0.5, cc[:], op.mult,
                                                 op.mult)
                  nc.vector.tensor_tensor(q1[:], q1[:], cc[:], op.add)
                  q1v = q1[:].rearrange("p (f k) -> p f k", k=4)
                  sl1s = smallp.tile([128, F], f32, tag="sl1s", name="sl1s")
                  nc.vector.tensor_reduce(sl1s[:], q1v, axis=X, op=op.add)

                  # ---- contributions ----
                  scr = smallp.tile([128, F], f32, tag="scr", name="scr")
                  nc.vector.scalar_tensor_tensor(
                      scr[:], posf[:], 1.0, lse[:], op.mult, op.mult,
                      accum_out=staging[:, t * NQ + 2: t * NQ + 3])
                  scr3 = smallp.tile([128, F], f32, tag="scr3", name="scr3")
                  nc.vector.scalar_tensor_tensor(
                      scr3[:], negf[:], 1.0, lse[:], op.mult, op.mult,
                      accum_out=staging[:, t * NQ + 4: t * NQ + 5])
                  scr4 = smallp.tile([128, F], f32, tag="scr4", name="scr4")
                  nc.vector.scalar_tensor_tensor(
                      scr4[:], negf[:], 1.0, cls0[:], op.mult, op.mult,
                      accum_out=staging[:, t * NQ + 5: t * NQ + 6])
                  scr5 = smallp.tile([128, F], f32, tag="scr5", name="scr5")
                  nc.vector.scalar_tensor_tensor(
                      scr5[:], posf[:], 1.0, sl1s[:], op.mult, op.mult,
                      accum_out=staging[:, t * NQ + 6: t * NQ + 7])

            import os as _os2
            gsb = constp.tile([M, C], f32, tag="gsb", name="gsb")
            if _os2.environ.get("SKIP_PE", "0") != "1":
                nc.scalar.activation(gsb[:], gmat_ps[:], act.Copy)
            else:
                nc.vector.memset(gsb[:], 0.0)
            nc.sync.dma_start(gmat_d[:], gsb[:])
            nc.sync.dma_start(stage_d[:], staging[:])

    nc.compile()
    return nc


def _combine(stages, gmats, labels_list):
    f32 = np.float32
    cls_losses, reg_losses, n_pos_list = [], [], []
    for st, gm, labs in zip(stages, gmats, labels_list):
        s = st.reshape(128, NT, NQ).sum(axis=(0, 1), dtype=np.float64)
        n_pos, n_neg = s[0], -s[1]
        plse, nlse, ncls0, psl1 = s[2], -s[4], -s[5], s[6]
        pcls = gm[np.arange(M), labs].sum(dtype=np.float64)
        cl = plse / max(n_pos, 1) - pcls / max(n_pos, 1) \
            + nlse / max(n_neg, 1) - ncls0 / max(n_neg, 1)
        rl = psl1 / max(4 * n_pos, 1)
        cls_losses.append(cl)
        reg_losses.append(rl)
        n_pos_list.append(n_pos)
    total_pos = int(round(sum(n_pos_list)))
    cls_final = f32(np.mean(np.array(cls_losses)))
    reg_final = f32(np.sum(np.array(reg_losses)) / max(total_pos, 1))
    total = f32(cls_final + reg_final)
    return total, cls_final, reg_final, np.int32(total_pos)


def kernel(cls_output, reg_output, anchors, target_boxes, target_labels):
    global LAST_RESULTS
    import os
    from concourse.bass_utils import run_bass_kernel_spmd

    if "nc" not in _CACHE:
        _CACHE["nc"] = _build_nc()
    nc = _CACHE["nc"]

    aca, acr, avv = _host_prep_shared(anchors)
    in_maps = []
    for i in range(B):
        clsb, regb, boxr, ptab = _host_prep_image(
            cls_output[i], reg_output[i], target_boxes[i], target_labels[i])
        in_maps.append(dict(clsb=clsb, regb=regb, aca=aca, acr=acr, avv=avv,
                            boxr=boxr, ptab=ptab))

    trace = os.environ.get("DETLOSS_TRACE", "0") == "1"
    res = run_bass_kernel_spmd(nc, in_maps, core_ids=list(range(B)), trace=trace)
    LAST_RESULTS = res
    stages = [r["stage"] for r in res.results]
    gmats = [r["gmat"] for r in res.results]
    labels_f = [np.asarray(target_labels[i]).astype(np.int64) for i in range(B)]
    return _combine(stages, gmats, labels_f)


if __name__ == "__main__":
    data = np.load("/root/problem/ref_inputs.npz")
    out = kernel(data["cls_output"], data["reg_output"], data["anchors"],
                 data["target_boxes"], data["target_labels"])
    print("kernel out:", [float(o) for o in out])



# revision 12
# speedup vs baseline: 1.8449x; 1.8449x over previous
"""Trainium2 Bass kernel for the detection-loss problem (v2, fp16).

Data-parallel over batch: each of 8 NeuronCores processes one image.

Per-core pipeline, all heavy tensors fp16 (validated vs f64: ~5e-4 rel err):
  stage A  decode anchors image-wide ([128, 720]-scale ops)
  pair     [128, (M=32, F=72)] tiles, f innermost so broadcast operands
           keep step-1 APs: min/max/sub chain -> w',h' packed (x||y),
           ir = min(w',0)*h', sab = areaA+areaB (GPSIMD), score
           lq = Ln(ir) - Ln(sab) (scalar engine Ln), 5-step pairwise
           tree max over m -> mx, multi-hot amask = is_eq(lq, mx)
           (ties averaged later via 1/cnt; pos-masked via mx*posf trick)
  payload  PE transposes amask 128-col chunks (strided (m,f_l) view),
           then block-diag matmul vs Pblk -> per-anchor payload
           (gcx, gcy, ln bw, ln bh, cnt) in PSUM
  gmat     PE: G[m,c] += amask_f^T @ cls_f, host reads G[m, label_m]
  CE       Exp (scalar) -> group-sum over classes (GPSIMD) -> Ln -> lse
  reg      targets from payload/cnt, smooth-L1, pos-masked, accum
  sums     stt accum_out -> staging [128, NT*8] -> DRAM; host combines

Anchor indexing: n = (t*F + f)*128 + p  (column-major), padded to 92160.
Class dim padded 21 -> 24 with -100 logits (exp -> 0).
Pad anchors are forced far away => always neg; host subtracts the
deterministic pad contributions (count and ln(21) lse each).
"""

import numpy as np

B, A, C, H, W = 8, 9, 21, 100, 100
M = 32
N = A * H * W            # 90000
F = 72                   # anchor columns per tile
NT = 10                  # tiles per image
NTF = NT * F             # 720
NPAD = 128 * NTF         # 92160
NPAD_EXTRA = NPAD - N    # 2160 pad anchors, always neg
CP = 24                  # padded class count
NQ = 8                   # staging quantities per tile
POS_THR, NEG_THR = 0.25, 0.1
LN_POS = float(np.log(POS_THR / (1.0 + POS_THR)))   # ln q threshold
LN_NEG = float(np.log(NEG_THR / (1.0 + NEG_THR)))
EPS_IR = 1e-4
PAD_CLS = -100.0

_CACHE = {}
LAST_RESULTS = None


def _col_tiled(arr_nk):
    """[NPAD, k] -> [128, NTF*k] fp16, anchor n=(col*128+p) at [p, col*k+j]."""
    k = arr_nk.shape[1]
    return (
        arr_nk.reshape(NTF, 128, k).transpose(1, 0, 2).reshape(128, NTF * k)
    ).astype(np.float16).copy()


def _pad_rows(x, pad_vals):
    out = np.empty((NPAD, x.shape[1]), np.float32)
    out[:N] = x
    out[N:] = pad_vals
    return out


def _host_prep_shared(anchors):
    anchors = np.asarray(anchors, np.float32)
    aw = anchors[:, 2] - anchors[:, 0]
    ah = anchors[:, 3] - anchors[:, 1]
    acx = anchors[:, 0] + 0.5 * aw
    acy = anchors[:, 1] + 0.5 * ah

    # stage-A constants: cx = r0*c1 + c2, HX = aw*exp(r2), dx0/1 = cx -/+ HX/2
    c1x, c2x = aw / 2, acx - aw / 4
    c1y, c2y = ah / 2, acy - ah / 4
    acon = _pad_rows(np.stack([aw, ah, c1x, c2x, c1y, c2y], 1),
                     [1.0, 1.0, 0.5, 8000.0, 0.5, 8000.0])
    acon_t = _col_tiled(acon)

    # reg-target affine: tgt = g*S + Bv (g = [gcx, gcy, ln bw, ln bh])
    S = np.stack([2 / aw, 2 / ah, np.ones(N, np.float32),
                  np.ones(N, np.float32)], 1)
    Bv = np.stack([0.5 - 2 * acx / aw, 0.5 - 2 * acy / ah,
                   -np.log(aw), -np.log(ah)], 1)
    s_t = _col_tiled(_pad_rows(S, [1.0, 1.0, 1.0, 1.0]))
    b_t = _col_tiled(_pad_rows(Bv, [0.0, 0.0, 0.0, 0.0]))
    ident = np.eye(128, dtype=np.float16).copy()
    return acon_t, s_t, b_t, ident


def _host_prep_image(cls_i, reg_i, tb_i, tl_i):
    f32 = np.float32
    # cls: [A,C,H,W] -> [N, C] with n = a*H*W + h*W + w  (matches reference
    # transpose(1,0,2,3).reshape(C,-1).T i.e. [A,H,W] flat order)
    cls_flat = np.transpose(np.asarray(cls_i, f32), (0, 2, 3, 1)).reshape(N, C)
    clsp = np.full((NPAD, CP), PAD_CLS, f32)
    clsp[:N, :C] = cls_flat
    clsp[N:, :C] = 0.0
    ct = _col_tiled(clsp)

    reg_flat = np.transpose(np.asarray(reg_i, f32), (0, 2, 3, 1)).reshape(N, 4)
    rt = _col_tiled(_pad_rows(reg_flat, [0.0, 0.0, 0.0, 0.0]))

    tb = np.asarray(tb_i, f32)
    bw = tb[:, 2] - tb[:, 0]
    bh = tb[:, 3] - tb[:, 1]
    bcx = tb[:, 0] + 0.5 * bw
    bcy = tb[:, 1] + 0.5 * bh
    areaB = bw * bh

    # box tensors expanded over f (m outer, f inner), replicated 128 partitions
    def mexp(vals_xy):  # list of [M] -> [128, len*M*F]
        cat = np.concatenate([np.repeat(v.astype(np.float16), F) for v in vals_xy])
        return np.tile(cat, (128, 1)).copy()

    bxy0 = mexp([tb[:, 0], tb[:, 1]])          # [128, 2*M*F]
    bxy1 = mexp([tb[:, 2], tb[:, 3]])
    areab = mexp([areaB])                      # [128, M*F]

    # payload block rhs: rows (fl*32 + m), cols (fl*8 + k)
    pay = np.stack([bcx, bcy, np.log(bw), np.log(bh), np.ones(M, f32),
                    np.zeros(M, f32), np.zeros(M, f32), np.zeros(M, f32)], 1)
    pblk = np.zeros((128, 32), np.float16)
    for m in range(M):
        for fl in range(4):
            pblk[fl * 32 + m, fl * 8:fl * 8 + 8] = pay[m].astype(np.float16)
    return ct, rt, bxy0, bxy1, areab, pblk.copy()


def _build_nc():
    import concourse.bacc as bacc
    import concourse.mybir as mybir
    from concourse.tile import TileContext

    dt = mybir.dt
    f16 = dt.float16
    f32 = dt.float32
    op = mybir.AluOpType
    act = mybir.ActivationFunctionType
    X = mybir.AxisListType.X

    nc = bacc.Bacc("TRN2", target_bir_lowering=False, debug=False, num_devices=8)

    ct_d = nc.dram_tensor("ct", [128, NTF * CP], f16, kind="ExternalInput")
    rt_d = nc.dram_tensor("rt", [128, NTF * 4], f16, kind="ExternalInput")
    acon_d = nc.dram_tensor("acon", [128, NTF * 6], f16, kind="ExternalInput")
    s_d = nc.dram_tensor("sv", [128, NTF * 4], f16, kind="ExternalInput")
    b_d = nc.dram_tensor("bv", [128, NTF * 4], f16, kind="ExternalInput")
    bxy0_d = nc.dram_tensor("bxy0", [128, 2 * M * F], f16, kind="ExternalInput")
    bxy1_d = nc.dram_tensor("bxy1", [128, 2 * M * F], f16, kind="ExternalInput")
    areab_d = nc.dram_tensor("areab", [128, M * F], f16, kind="ExternalInput")
    pblk_d = nc.dram_tensor("pblk", [128, 32], f16, kind="ExternalInput")
    ident_d = nc.dram_tensor("ident", [128, 128], f16, kind="ExternalInput")
    stage_d = nc.dram_tensor("stage", [128, NT * NQ], f32, kind="ExternalOutput")
    gmat_d = nc.dram_tensor("gmat", [M, CP], f32, kind="ExternalOutput")

    MF = M * F           # 2304
    NCHUNK = MF // 128   # 18 transpose chunks per tile
    GRP = 4              # chunks per transpose group (PSUM bank sized)

    with TileContext(nc) as tc:
        with (
            tc.tile_pool(name="const", bufs=1) as constp,
            tc.tile_pool(name="anc", bufs=1) as ancp,
            tc.tile_pool(name="clsp", bufs=2) as clsp,
            tc.tile_pool(name="pair", bufs=2) as pairp,
            tc.tile_pool(name="tree", bufs=2) as treep,
            tc.tile_pool(name="small", bufs=3) as smallp,
            tc.tile_pool(name="regs", bufs=2) as regsp,
            tc.tile_pool(name="maskt", bufs=3) as masktp,
            tc.tile_pool(name="pst", bufs=3, space="PSUM") as pstp,
            tc.tile_pool(name="psg", bufs=2, space="PSUM") as psgp,
            tc.tile_pool(name="psgm", bufs=1, space="PSUM") as psgmp,
        ):
            # ---- load constants / inputs ----
            def cload(name, shape, dram):
                t = constp.tile(shape, f16, tag=name, name=name)
                nc.sync.dma_start(t[:], dram[:])
                return t

            bxy0e = cload("bxy0e", [128, 2 * MF], bxy0_d)
            bxy1e = cload("bxy1e", [128, 2 * MF], bxy1_d)
            areabe = cload("areabe", [128, MF], areab_d)
            pblk = cload("pblk", [128, 32], pblk_d)
            ident = cload("ident", [128, 128], ident_d)
            acon = cload("acon", [128, NTF * 6], acon_d)
            sall = cload("sall", [128, NTF * 4], s_d)
            ball = cload("ball", [128, NTF * 4], b_d)
            regall = cload("regall", [128, NTF * 4], rt_d)

            staging = constp.tile([128, NT * NQ], f32, tag="staging", name="staging")
            nc.vector.memset(staging[:], 0.0)
            gmat_ps = psgmp.tile([M, CP], f32, tag="gmat_ps", name="gmat_ps")

            rview = regall[:].rearrange("p (n k) -> p n k", k=4)
            aview = acon[:].rearrange("p (n k) -> p n k", k=6)
            sview = sall[:].rearrange("p (n k) -> p n k", k=4)
            bview = ball[:].rearrange("p (n k) -> p n k", k=4)

            # ---- stage A: decode all anchors ----
            # dxy0/dxy1: [128, 2*NTF] (x block then y block); areaA: [128, NTF]
            dxy0 = ancp.tile([128, 2 * NTF], f16, tag="dxy0", name="dxy0")
            dxy1 = ancp.tile([128, 2 * NTF], f16, tag="dxy1", name="dxy1")
            areaa = ancp.tile([128, NTF], f16, tag="areaa", name="areaa")
            with tc.tile_pool(name="tmpa", bufs=1) as tmpap:
                ewh = tmpap.tile([128, 2 * NTF], f16, tag="ewh", name="ewh")
                ewh_v = ewh[:].rearrange("p (xy n) -> p xy n", xy=2)
                # exp(r2), exp(r3) -> ewh[0], ewh[1]
                nc.scalar.activation(ewh_v[:, 0, :], rview[:, :, 2], act.Exp)
                nc.scalar.activation(ewh_v[:, 1, :], rview[:, :, 3], act.Exp)
                hxy = tmpap.tile([128, 2 * NTF], f16, tag="hxy", name="hxy")
                hxy_v = hxy[:].rearrange("p (xy n) -> p xy n", xy=2)
                nc.vector.tensor_tensor(hxy_v[:, 0, :], ewh_v[:, 0, :],
                                        aview[:, :, 0], op.mult)
                nc.vector.tensor_tensor(hxy_v[:, 1, :], ewh_v[:, 1, :],
                                        aview[:, :, 1], op.mult)
                cxy = tmpap.tile([128, 2 * NTF], f16, tag="cxy", name="cxy")
                cxy_v = cxy[:].rearrange("p (xy n) -> p xy n", xy=2)
                nc.vector.tensor_tensor(cxy_v[:, 0, :], rview[:, :, 0],
                                        aview[:, :, 2], op.mult)
                nc.vector.tensor_tensor(cxy_v[:, 0, :], cxy_v[:, 0, :],
                                        aview[:, :, 3], op.add)
                nc.vector.tensor_tensor(cxy_v[:, 1, :], rview[:, :, 1],
                                        aview[:, :, 4], op.mult)
                nc.vector.tensor_tensor(cxy_v[:, 1, :], cxy_v[:, 1, :],
                                        aview[:, :, 5], op.add)
                # dxy0 = cxy - 0.5*hxy ; dxy1 = cxy + 0.5*hxy (packed xy)
                nc.vector.scalar_tensor_tensor(dxy0[:], hxy[:], -0.5, cxy[:],
                                               op.mult, op.add)
                nc.vector.scalar_tensor_tensor(dxy1[:], hxy[:], 0.5, cxy[:],
                                               op.mult, op.add)
                nc.vector.tensor_tensor(areaa[:], hxy_v[:, 0, :], hxy_v[:, 1, :],
                                        op.mult)

            dxy0_v = dxy0[:].rearrange("p (xy n) -> p xy n", xy=2)
            dxy1_v = dxy1[:].rearrange("p (xy n) -> p xy n", xy=2)

            P2MF = [128, 2, M, F]
            PMF = [128, M, F]

            # ---- per-tile main loop ----
            for t in range(NT):
                tf = slice(t * F, (t + 1) * F)

                def b_anc(v, tag=None):  # [128, 2, F] -> [128, 2, M, F]
                    return v[:, :, tf].unsqueeze(2).broadcast_to(P2MF)

                txy1 = pairp.tile([128, 2 * MF], f16, tag="txy1", name="txy1")
                txy1v = txy1[:].rearrange("p (xy m f) -> p xy m f", xy=2, m=M)
                bxy1v = bxy1e[:].rearrange("p (xy m f) -> p xy m f", xy=2, m=M)
                nc.vector.tensor_tensor(txy1v, b_anc(dxy1_v), bxy1v, op.min)

                nwxy = pairp.tile([128, 2 * MF], f16, tag="nwxy", name="nwxy")
                nwxyv = nwxy[:].rearrange("p (xy m f) -> p xy m f", xy=2, m=M)
                bxy0v = bxy0e[:].rearrange("p (xy m f) -> p xy m f", xy=2, m=M)
                nc.vector.tensor_tensor(nwxyv, b_anc(dxy0_v), bxy0v, op.max)
                # whp = nwxy - txy1  (>0 means no overlap on that axis)
                nc.vector.tensor_tensor(nwxy[:], nwxy[:], txy1[:], op.subtract)

                ir = pairp.tile([128, MF], f16, tag="ir", name="ir")
                wp = nwxy[:, 0:MF]
                hp = nwxy[:, MF:2 * MF]
                # ir = min(wp,0)*hp : = w*h>0 iff both axes overlap
                nc.vector.scalar_tensor_tensor(ir[:], wp, 0.0, hp, op.min, op.mult)
                nc.vector.tensor_scalar(ir[:], ir[:], EPS_IR, None, op.max)

                sab = pairp.tile([128, MF], f16, tag="sab", name="sab")
                av = areaa[:, tf].unsqueeze(1).broadcast_to(PMF)
                abv = areabe[:].rearrange("p (m f) -> p m f", m=M)
                nc.gpsimd.tensor_tensor(
                    sab[:].rearrange("p (m f) -> p m f", m=M), av, abv, op.add)

                nc.scalar.activation(ir[:], ir[:], act.Ln)
                nc.scalar.activation(sab[:], sab[:], act.Ln)
                lq = ir
                nc.vector.tensor_tensor(lq[:], lq[:], sab[:], op.subtract)

                # tree max over m (5 halvings)
                lqv = lq[:].rearrange("p (m f) -> p m f", m=M)
                tr16 = treep.tile([128, 16 * F], f16, tag="tr16", name="tr16")
                t16v = tr16[:].rearrange("p (m f) -> p m f", m=16)
                nc.vector.tensor_tensor(t16v, lqv[:, 0:16, :], lqv[:, 16:32, :], op.max)
                tr8 = treep.tile([128, 8 * F], f16, tag="tr8", name="tr8")
                t8v = tr8[:].rearrange("p (m f) -> p m f", m=8)
                nc.vector.tensor_tensor(t8v, t16v[:, 0:8, :], t16v[:, 8:16, :], op.max)
                tr4 = treep.tile([128, 4 * F], f16, tag="tr4", name="tr4")
                t4v = tr4[:].rearrange("p (m f) -> p m f", m=4)
                nc.vector.tensor_tensor(t4v, t8v[:, 0:4, :], t8v[:, 4:8, :], op.max)
                tr2 = treep.tile([128, 2 * F], f16, tag="tr2", name="tr2")
                t2v = tr2[:].rearrange("p (m f) -> p m f", m=2)
                nc.vector.tensor_tensor(t2v, t4v[:, 0:2, :], t4v[:, 2:4, :], op.max)
                mx = smallp.tile([128, F], f16, tag="mx", name="mx")
                nc.vector.tensor_tensor(mx[:], t2v[:, 0, :], t2v[:, 1, :], op.max)

                posf = smallp.tile([128, F], f16, tag="posf", name="posf")
                nc.vector.tensor_scalar(posf[:], mx[:], LN_POS, None, op.is_ge)
                negf = smallp.tile([128, F], f16, tag="negf", name="negf")
                nc.vector.tensor_scalar(negf[:], mx[:], LN_NEG, None, op.is_lt)
                # mxp = mx*posf + (1-posf)*1e4 : equals mx exactly on pos rows
                u1 = smallp.tile([128, F], f16, tag="u1", name="u1")
                nc.vector.tensor_scalar(u1[:], posf[:], -1e4, 1e4, op.mult, op.add)
                mxp = smallp.tile([128, F], f16, tag="mxp", name="mxp")
                nc.vector.tensor_tensor(mxp[:], mx[:], posf[:], op.mult)
                nc.vector.tensor_tensor(mxp[:], mxp[:], u1[:], op.add)

                # amask stored (f outer, m inner) so PE-transpose chunks are
                # contiguous; the strided write makes this pass 1x, accepted.
                amask = pairp.tile([128, MF], f16, tag="amask", name="amask")
                amv = amask[:].rearrange("p (f m) -> p m f", m=M)
                nc.vector.tensor_tensor(
                    amv, lqv, mxp[:].unsqueeze(1).broadcast_to(PMF), op.is_ge)

                # ---- CE ----
                ctt = clsp.tile([128, F * CP], f16, tag="ctt", name="ctt")
                nc.sync.dma_start(ctt[:], ct_d[:, t * F * CP:(t + 1) * F * CP])
                ctv = ctt[:].rearrange("p (f c) -> p f c", c=CP)
                ett = clsp.tile([128, F * CP], f16, tag="ett", name="ett")
                nc.scalar.activation(ett[:], ctt[:], act.Exp)
                etv = ett[:].rearrange("p (f c) -> p f c", c=CP)
                # class-sum as pairwise tree on GPSIMD (DVE stays on pair work)
                e12 = treep.tile([128, F * 12], f16, tag="e12", name="e12")
                e12v = e12[:].rearrange("p (f c) -> p f c", c=12)
                nc.gpsimd.tensor_tensor(e12v, etv[:, :, 0:12], etv[:, :, 12:24],
                                        op.add)
                e6 = treep.tile([128, F * 6], f16, tag="e6", name="e6")
                e6v = e6[:].rearrange("p (f c) -> p f c", c=6)
                nc.gpsimd.tensor_tensor(e6v, e12v[:, :, 0:6], e12v[:, :, 6:12],
                                        op.add)
                e3 = treep.tile([128, F * 3], f16, tag="e3", name="e3")
                e3v = e3[:].rearrange("p (f c) -> p f c", c=3)
                nc.gpsimd.tensor_tensor(e3v, e6v[:, :, 0:3], e6v[:, :, 3:6],
                                        op.add)
                se = smallp.tile([128, F], f32, tag="se", name="se")
                nc.gpsimd.tensor_tensor(se[:], e3v[:, :, 0], e3v[:, :, 1], op.add)
                nc.gpsimd.tensor_tensor(se[:], se[:], e3v[:, :, 2], op.add)
                lse = smallp.tile([128, F], f16, tag="lse", name="lse")
                nc.scalar.activation(lse[:], se[:], act.Ln)
                cls0 = smallp.tile([128, F], f16, tag="cls0", name="cls0")
                nc.gpsimd.tensor_copy(cls0[:], ctv[:, :, 0])

                # ---- counts + CE sums into staging ----
                scr = smallp.tile([128, F], f16, tag="scr", name="scr")
                nc.vector.tensor_scalar(
                    scr[:], posf[:], 1.0, 0.0, op.mult, op.add,
                    accum_out=staging[:, t * NQ + 0: t * NQ + 1])
                scr1 = smallp.tile([128, F], f16, tag="scr1", name="scr1")
                nc.vector.tensor_scalar(
                    scr1[:], negf[:], 1.0, 0.0, op.mult, op.add,
                    accum_out=staging[:, t * NQ + 1: t * NQ + 2])
                scr2 = smallp.tile([128, F], f16, tag="scr2", name="scr2")
                nc.vector.scalar_tensor_tensor(
                    scr2[:], posf[:], 1.0, lse[:], op.mult, op.mult,
                    accum_out=staging[:, t * NQ + 2: t * NQ + 3])
                scr3 = smallp.tile([128, F], f16, tag="scr3", name="scr3")
                nc.vector.scalar_tensor_tensor(
                    scr3[:], negf[:], 1.0, lse[:], op.mult, op.mult,
                    accum_out=staging[:, t * NQ + 3: t * NQ + 4])
                scr4 = smallp.tile([128, F], f16, tag="scr4", name="scr4")
                nc.vector.scalar_tensor_tensor(
                    scr4[:], negf[:], 1.0, cls0[:], op.mult, op.mult,
                    accum_out=staging[:, t * NQ + 4: t * NQ + 5])

                # ---- PE: transpose amask chunks + payload matmuls ----
                g_ps = psgp.tile([128, NCHUNK * 32], f32, tag="g_ps", name="g_ps")
                done = 0
                while done < NCHUNK:
                    gn = min(GRP, NCHUNK - done)
                    ps_t = pstp.tile([128, GRP * 128], f16, tag="ps_t", name="ps_t")
                    for c in range(gn):
                        ch = done + c
                        inv = amask[:, ch * 128:(ch + 1) * 128]
                        nc.tensor.transpose(
                            ps_t[:, c * 128:(c + 1) * 128], inv, ident[:])
                    amt = masktp.tile([128, GRP * 128], f16, tag="amt", name="amt")
                    nc.scalar.activation(amt[:, 0:gn * 128], ps_t[:, 0:gn * 128],
                                         act.Copy)
                    for c in range(gn):
                        ch = done + c
                        nc.tensor.matmul(
                            g_ps[:, ch * 32:(ch + 1) * 32],
                            amt[:, c * 128:(c + 1) * 128], pblk[:],
                            start=True, stop=True)
                    done += gn

                gsb = regsp.tile([128, NCHUNK * 32], f16, tag="gsb", name="gsb")
                nc.scalar.activation(gsb[:], g_ps[:], act.Copy)
                gv = gsb[:].rearrange("p (f k) -> p f k", k=8)

                # ---- gmat: G[m,c] += amask_f^T @ cls_f ----
                for f in range(F):
                    nc.tensor.matmul(
                        gmat_ps[:, :], amask[:, f * M:(f + 1) * M], ctv[:, f, :],
                        start=(t == 0 and f == 0),
                        stop=(t == NT - 1 and f == F - 1))

                # ---- reg loss ----
                cntc = smallp.tile([128, F], f16, tag="cntc", name="cntc")
                nc.vector.tensor_scalar(cntc[:], gv[:, :, 4], 1.0, None, op.max)
                rc = smallp.tile([128, F], f32, tag="rc", name="rc")
                nc.vector.reciprocal(rc[:], cntc[:])

                def rtile(tag):
                    return regsp.tile([128, F * 4], f16, tag=tag, name=tag)

                t1 = rtile("t1")
                t1v = t1[:].rearrange("p (f k) -> p f k", k=4)
                nc.vector.tensor_tensor(
                    t1v, gv[:, :, 0:4],
                    rc[:].unsqueeze(2).broadcast_to([128, F, 4]), op.mult)
                nc.vector.tensor_tensor(t1v, t1v, sview[:, tf, :], op.mult)
                dd = rtile("dd")
                ddv = dd[:].rearrange("p (f k) -> p f k", k=4)
                nc.vector.tensor_tensor(ddv, rview[:, tf, :], bview[:, tf, :],
                                        op.subtract)
                nc.vector.tensor_tensor(dd[:], dd[:], t1[:], op.subtract)
                ad = rtile("ad")
                nc.scalar.activation(ad[:], dd[:], act.Abs)
                cc = rtile("cc")
                ccv = cc[:].rearrange("p (f k) -> p f k", k=4)
                nc.vector.scalar_tensor_tensor(
                    ccv, ad[:].rearrange("p (f k) -> p f k", k=4), 1.0,
                    posf[:].unsqueeze(2).broadcast_to([128, F, 4]),
                    op.min, op.mult)
                t3 = rtile("t3")
                nc.vector.scalar_tensor_tensor(t3[:], cc[:], -0.5, ad[:],
                                               op.mult, op.add)
                sl1 = rtile("sl1")
                nc.vector.scalar_tensor_tensor(
                    sl1[:], cc[:], 1.0, t3[:], op.mult, op.mult,
                    accum_out=staging[:, t * NQ + 5: t * NQ + 6])

            gout = constp.tile([M, CP], f32, tag="gout", name="gout")
            nc.scalar.activation(gout[:], gmat_ps[:], act.Copy)
            nc.sync.dma_start(gmat_d[:], gout[:])
            nc.sync.dma_start(stage_d[:], staging[:])

    nc.compile()
    return nc


def _combine(stages, gmats, labels_list):
    cls_losses, reg_losses, n_pos_list = [], [], []
    ln21 = np.log(21.0)
    for st, gm, labs in zip(stages, gmats, labels_list):
        s = st.reshape(128, NT, NQ).sum(axis=(0, 1), dtype=np.float64)
        n_pos = s[0]
        n_neg = s[1] - NPAD_EXTRA
        plse = s[2]
        nlse = s[3] - NPAD_EXTRA * ln21
        ncls0 = s[4]
        psl1 = s[5]
        pcls = gm[np.arange(M), labs].sum(dtype=np.float64)
        cl = (plse - pcls) / max(n_pos, 1) + (nlse - ncls0) / max(n_neg, 1)
        rl = psl1 / max(4 * n_pos, 1)
        cls_losses.append(cl)
        reg_losses.append(rl)
        n_pos_list.append(n_pos)
    total_pos = int(round(sum(n_pos_list)))
    cls_final = np.float32(np.mean(np.array(cls_losses)))
    reg_final = np.float32(np.sum(np.array(reg_losses)) / max(total_pos, 1))
    total = np.float32(cls_final + reg_final)
    return total, cls_final, reg_final, np.int32(total_pos)


def kernel(cls_output, reg_output, anchors, target_boxes, target_labels):
    global LAST_RESULTS
    import os
    from concourse.bass_utils import run_bass_kernel_spmd

    if "nc" not in _CACHE:
        _CACHE["nc"] = _build_nc()
    nc = _CACHE["nc"]

    acon_t, s_t, b_t, ident = _host_prep_shared(anchors)
    in_maps = []
    for i in range(B):
        ct, rt, bxy0, bxy1, areab, pblk = _host_prep_image(
            cls_output[i], reg_output[i], target_boxes[i], target_labels[i])
        in_maps.append(dict(ct=ct, rt=rt, acon=acon_t, sv=s_t, bv=b_t,
                            bxy0=bxy0, bxy1=bxy1, areab=areab, pblk=pblk,
                            ident=ident))

    trace = os.environ.get("DETLOSS_TRACE", "0") == "1"
    res = run_bass_kernel_spmd(nc, in_maps, core_ids=list(range(B)), trace=trace)
    LAST_RESULTS = res
    stages = [r["stage"] for r in res.results]
    gmats = [r["gmat"] for r in res.results]
    labels_f = [np.asarray(target_labels[i]).astype(np.int64) for i in range(B)]
    return _combine(stages, gmats, labels_f)


if __name__ == "__main__":
    data = np.load("/root/problem/ref_inputs.npz")
    out = kernel(data["cls_output"], data["reg_output"], data["anchors"],
                 data["target_boxes"], data["target_labels"])
    print("kernel out:", [float(o) for o in out])


# revision 20
# speedup vs baseline: 2.1580x; 1.1697x over previous
"""Trainium2 Bass kernel for the detection-loss problem (v2, fp16).

Data-parallel over batch: each of 8 NeuronCores processes one image.

Per-core pipeline, all heavy tensors fp16 (validated vs f64: ~5e-4 rel err):
  stage A  decode anchors image-wide ([128, 720]-scale ops)
  pair     [128, (M=32, F=72)] tiles, f innermost so broadcast operands
           keep step-1 APs: min/max/sub chain -> w',h' packed (x||y),
           ir = min(w',0)*h', sab = areaA+areaB (GPSIMD), score
           lq = Ln(ir) - Ln(sab) (scalar engine Ln), 5-step pairwise
           tree max over m -> mx, multi-hot amask = is_eq(lq, mx)
           (ties averaged later via 1/cnt; pos-masked via mx*posf trick)
  payload  PE transposes amask 128-col chunks (strided (m,f_l) view),
           then block-diag matmul vs Pblk -> per-anchor payload
           (gcx, gcy, ln bw, ln bh, cnt) in PSUM
  gmat     PE: G[m,c] += amask_f^T @ cls_f, host reads G[m, label_m]
  CE       Exp (scalar) -> group-sum over classes (GPSIMD) -> Ln -> lse
  reg      targets from payload/cnt, smooth-L1, pos-masked, accum
  sums     stt accum_out -> staging [128, NT*8] -> DRAM; host combines

Anchor indexing: n = (t*F + f)*128 + p  (column-major), padded to 92160.
Class dim padded 21 -> 24 with -100 logits (exp -> 0).
Pad anchors are forced far away => always neg; host subtracts the
deterministic pad contributions (count and ln(21) lse each).
"""

import numpy as np

B, A, C, H, W = 8, 9, 21, 100, 100
M = 32
N = A * H * W            # 90000
F = 72                   # anchor columns per tile
NT = 10                  # tiles per image
NTF = NT * F             # 720
NPAD = 128 * NTF         # 92160
NPAD_EXTRA = NPAD - N    # 2160 pad anchors, always neg
CP = 24                  # padded class count
NQ = 8                   # staging quantities per tile
POS_THR, NEG_THR = 0.25, 0.1
LN_POS = float(np.log(POS_THR / (1.0 + POS_THR)))   # ln q threshold
LN_NEG = float(np.log(NEG_THR / (1.0 + NEG_THR)))
EPS_IR = 1e-4
PAD_CLS = -100.0

_CACHE = {}
LAST_RESULTS = None


def _col_tiled(arr_nk):
    """[NPAD, k] -> [128, NTF*k] fp16, anchor n=(col*128+p) at [p, col*k+j]."""
    k = arr_nk.shape[1]
    return (
        arr_nk.reshape(NTF, 128, k).transpose(1, 0, 2).reshape(128, NTF * k)
    ).astype(np.float16).copy()


def _pad_rows(x, pad_vals):
    out = np.empty((NPAD, x.shape[1]), np.float32)
    out[:N] = x
    out[N:] = pad_vals
    return out


def _host_prep_shared(anchors):
    anchors = np.asarray(anchors, np.float32)
    aw = anchors[:, 2] - anchors[:, 0]
    ah = anchors[:, 3] - anchors[:, 1]
    acx = anchors[:, 0] + 0.5 * aw
    acy = anchors[:, 1] + 0.5 * ah

    # stage-A constants: cx = r0*c1 + c2, HX = aw*exp(r2), dx0/1 = cx -/+ HX/2
    c1x, c2x = aw / 2, acx - aw / 4
    c1y, c2y = ah / 2, acy - ah / 4
    acon = _pad_rows(np.stack([aw, ah, c1x, c2x, c1y, c2y], 1),
                     [1.0, 1.0, 0.5, 8000.0, 0.5, 8000.0])
    acon_t = _col_tiled(acon)

    # reg-target affine: tgt = g*S + Bv (g = [gcx, gcy, ln bw, ln bh])
    S = np.stack([2 / aw, 2 / ah, np.ones(N, np.float32),
                  np.ones(N, np.float32)], 1)
    Bv = np.stack([0.5 - 2 * acx / aw, 0.5 - 2 * acy / ah,
                   -np.log(aw), -np.log(ah)], 1)
    s_t = _col_tiled(_pad_rows(S, [1.0, 1.0, 1.0, 1.0]))
    b_t = _col_tiled(_pad_rows(Bv, [0.0, 0.0, 0.0, 0.0]))
    ident = np.eye(128, dtype=np.float16).copy()
    return acon_t, s_t, b_t, ident


def _host_prep_image(cls_i, reg_i, tb_i, tl_i):
    f32 = np.float32
    # cls: [A,C,H,W] -> [N, C] with n = a*H*W + h*W + w  (matches reference
    # transpose(1,0,2,3).reshape(C,-1).T i.e. [A,H,W] flat order)
    cls_flat = np.transpose(np.asarray(cls_i, f32), (0, 2, 3, 1)).reshape(N, C)
    clsp = np.full((NPAD, CP), PAD_CLS, f32)
    clsp[:N, :C] = cls_flat
    clsp[N:, :C] = 0.0
    # per-tile blocks laid out (c outer, f inner): [p, t*CP*F + c*F + f]
    ct = (clsp.reshape(NT, F, 128, CP).transpose(2, 0, 3, 1)
          .reshape(128, NT * CP * F)).astype(np.float16).copy()

    reg_flat = np.transpose(np.asarray(reg_i, f32), (0, 2, 3, 1)).reshape(N, 4)
    rt = _col_tiled(_pad_rows(reg_flat, [0.0, 0.0, 0.0, 0.0]))

    tb = np.asarray(tb_i, f32)
    bw = tb[:, 2] - tb[:, 0]
    bh = tb[:, 3] - tb[:, 1]
    bcx = tb[:, 0] + 0.5 * bw
    bcy = tb[:, 1] + 0.5 * bh
    areaB = bw * bh

    # box tensors expanded over f (m outer, f inner), replicated 128 partitions
    def mexp(vals_xy):  # list of [M] -> [128, len*M*F]
        cat = np.concatenate([np.repeat(v.astype(np.float16), F) for v in vals_xy])
        return np.tile(cat, (128, 1)).copy()

    bxy0 = mexp([tb[:, 0], tb[:, 1]])          # [128, 2*M*F]
    bxy1 = mexp([tb[:, 2], tb[:, 3]])
    areab = mexp([areaB])                      # [128, M*F]

    # payload block rhs: rows (fl*32 + m), cols (fl*8 + k)
    pay = np.stack([bcx, bcy, np.log(bw), np.log(bh), np.ones(M, f32),
                    np.zeros(M, f32), np.zeros(M, f32), np.zeros(M, f32)], 1)
    pblk = np.zeros((128, 32), np.float16)
    for m in range(M):
        for fl in range(4):
            pblk[fl * 32 + m, fl * 8:fl * 8 + 8] = pay[m].astype(np.float16)
    return ct, rt, bxy0, bxy1, areab, pblk.copy()


def _build_nc():
    import concourse.bacc as bacc
    import concourse.mybir as mybir
    from concourse.tile import TileContext

    dt = mybir.dt
    f16 = dt.float16
    f32 = dt.float32
    op = mybir.AluOpType
    act = mybir.ActivationFunctionType
    X = mybir.AxisListType.X

    nc = bacc.Bacc("TRN2", target_bir_lowering=False, debug=False, num_devices=8)

    ct_d = nc.dram_tensor("ct", [128, NTF * CP], f16, kind="ExternalInput")
    rt_d = nc.dram_tensor("rt", [128, NTF * 4], f16, kind="ExternalInput")
    acon_d = nc.dram_tensor("acon", [128, NTF * 6], f16, kind="ExternalInput")
    s_d = nc.dram_tensor("sv", [128, NTF * 4], f16, kind="ExternalInput")
    b_d = nc.dram_tensor("bv", [128, NTF * 4], f16, kind="ExternalInput")
    bxy0_d = nc.dram_tensor("bxy0", [128, 2 * M * F], f16, kind="ExternalInput")
    bxy1_d = nc.dram_tensor("bxy1", [128, 2 * M * F], f16, kind="ExternalInput")
    areab_d = nc.dram_tensor("areab", [128, M * F], f16, kind="ExternalInput")
    pblk_d = nc.dram_tensor("pblk", [128, 32], f16, kind="ExternalInput")
    ident_d = nc.dram_tensor("ident", [128, 128], f16, kind="ExternalInput")
    stage_d = nc.dram_tensor("stage", [128, NT * NQ], f32, kind="ExternalOutput")
    gmat_d = nc.dram_tensor("gmat", [M, CP], f32, kind="ExternalOutput")

    MF = M * F           # 2304
    NCHUNK = MF // 128   # 18 transpose chunks per tile
    GRP = 4              # chunks per transpose group (PSUM bank sized)

    with TileContext(nc) as tc:
        with (
            tc.tile_pool(name="const", bufs=1) as constp,
            tc.tile_pool(name="anc", bufs=1) as ancp,
            tc.tile_pool(name="clsp", bufs=2) as clsp,
            tc.tile_pool(name="pair", bufs=2) as pairp,
            tc.tile_pool(name="tree", bufs=2) as treep,
            tc.tile_pool(name="small", bufs=3) as smallp,
            tc.tile_pool(name="regs", bufs=2) as regsp,
            tc.tile_pool(name="maskt", bufs=3) as masktp,
            tc.tile_pool(name="pst", bufs=3, space="PSUM") as pstp,
            tc.tile_pool(name="psg", bufs=2, space="PSUM") as psgp,
            tc.tile_pool(name="psgm", bufs=1, space="PSUM") as psgmp,
        ):
            # ---- load constants / inputs ----
            def cload(name, shape, dram):
                t = constp.tile(shape, f16, tag=name, name=name)
                nc.sync.dma_start(t[:], dram[:])
                return t

            bxy0e = cload("bxy0e", [128, 2 * MF], bxy0_d)
            bxy1e = cload("bxy1e", [128, 2 * MF], bxy1_d)
            areabe = cload("areabe", [128, MF], areab_d)
            pblk = cload("pblk", [128, 32], pblk_d)
            ident = cload("ident", [128, 128], ident_d)
            acon = cload("acon", [128, NTF * 6], acon_d)
            sall = cload("sall", [128, NTF * 4], s_d)
            ball = cload("ball", [128, NTF * 4], b_d)
            regall = cload("regall", [128, NTF * 4], rt_d)

            staging = constp.tile([128, NT * NQ], f32, tag="staging", name="staging")
            nc.vector.memset(staging[:], 0.0)
            gmat_ps = psgmp.tile([M, CP], f32, tag="gmat_ps", name="gmat_ps")

            rview = regall[:].rearrange("p (n k) -> p n k", k=4)
            aview = acon[:].rearrange("p (n k) -> p n k", k=6)
            sview = sall[:].rearrange("p (n k) -> p n k", k=4)
            bview = ball[:].rearrange("p (n k) -> p n k", k=4)

            # ---- stage A: decode all anchors ----
            # dxy0/dxy1: [128, 2*NTF] (x block then y block); areaA: [128, NTF]
            dxy0 = ancp.tile([128, 2 * NTF], f16, tag="dxy0", name="dxy0")
            dxy1 = ancp.tile([128, 2 * NTF], f16, tag="dxy1", name="dxy1")
            areaa = ancp.tile([128, NTF], f16, tag="areaa", name="areaa")
            with tc.tile_pool(name="tmpa", bufs=1) as tmpap:
                ewh = tmpap.tile([128, 2 * NTF], f16, tag="ewh", name="ewh")
                ewh_v = ewh[:].rearrange("p (xy n) -> p xy n", xy=2)
                # exp(r2), exp(r3) -> ewh[0], ewh[1]
                nc.scalar.activation(ewh_v[:, 0, :], rview[:, :, 2], act.Exp)
                nc.scalar.activation(ewh_v[:, 1, :], rview[:, :, 3], act.Exp)
                hxy = tmpap.tile([128, 2 * NTF], f16, tag="hxy", name="hxy")
                hxy_v = hxy[:].rearrange("p (xy n) -> p xy n", xy=2)
                nc.vector.tensor_tensor(hxy_v[:, 0, :], ewh_v[:, 0, :],
                                        aview[:, :, 0], op.mult)
                nc.vector.tensor_tensor(hxy_v[:, 1, :], ewh_v[:, 1, :],
                                        aview[:, :, 1], op.mult)
                cxy = tmpap.tile([128, 2 * NTF], f16, tag="cxy", name="cxy")
                cxy_v = cxy[:].rearrange("p (xy n) -> p xy n", xy=2)
                nc.vector.tensor_tensor(cxy_v[:, 0, :], rview[:, :, 0],
                                        aview[:, :, 2], op.mult)
                nc.vector.tensor_tensor(cxy_v[:, 0, :], cxy_v[:, 0, :],
                                        aview[:, :, 3], op.add)
                nc.vector.tensor_tensor(cxy_v[:, 1, :], rview[:, :, 1],
                                        aview[:, :, 4], op.mult)
                nc.vector.tensor_tensor(cxy_v[:, 1, :], cxy_v[:, 1, :],
                                        aview[:, :, 5], op.add)
                # dxy0 = cxy - 0.5*hxy ; dxy1 = cxy + 0.5*hxy (packed xy)
                nc.vector.scalar_tensor_tensor(dxy0[:], hxy[:], -0.5, cxy[:],
                                               op.mult, op.add)
                nc.vector.scalar_tensor_tensor(dxy1[:], hxy[:], 0.5, cxy[:],
                                               op.mult, op.add)
                nc.vector.tensor_tensor(areaa[:], hxy_v[:, 0, :], hxy_v[:, 1, :],
                                        op.mult)

            dxy0_v = dxy0[:].rearrange("p (xy n) -> p xy n", xy=2)
            dxy1_v = dxy1[:].rearrange("p (xy n) -> p xy n", xy=2)

            P2MF = [128, 2, M, F]
            PMF = [128, M, F]

            # ---- per-tile main loop ----
            for t in range(NT):
                tf = slice(t * F, (t + 1) * F)

                def b_anc(v, tag=None):  # [128, 2, F] -> [128, 2, M, F]
                    return v[:, :, tf].unsqueeze(2).broadcast_to(P2MF)

                txy1 = pairp.tile([128, 2 * MF], f16, tag="txy1", name="txy1")
                txy1v = txy1[:].rearrange("p (xy m f) -> p xy m f", xy=2, m=M)
                bxy1v = bxy1e[:].rearrange("p (xy m f) -> p xy m f", xy=2, m=M)
                nc.vector.tensor_tensor(txy1v, b_anc(dxy1_v), bxy1v, op.min)

                nwxy = pairp.tile([128, 2 * MF], f16, tag="nwxy", name="nwxy")
                nwxyv = nwxy[:].rearrange("p (xy m f) -> p xy m f", xy=2, m=M)
                bxy0v = bxy0e[:].rearrange("p (xy m f) -> p xy m f", xy=2, m=M)
                nc.vector.tensor_tensor(nwxyv, b_anc(dxy0_v), bxy0v, op.max)
                # whp = nwxy - txy1  (>0 means no overlap on that axis)
                nc.vector.tensor_tensor(nwxy[:], nwxy[:], txy1[:], op.subtract)

                wp = nwxy[:, 0:MF]
                hp = nwxy[:, MF:2 * MF]
                # clamp wp to <=0 in place (4x ts), then ir = wp*hp (2x tt):
                # ir = w*h > 0 iff both axes overlap, <= 0 otherwise
                nc.vector.tensor_scalar(wp, wp, 0.0, None, op.min)
                ir = pairp.tile([128, MF], f16, tag="ir", name="ir")
                nc.vector.tensor_tensor(ir[:], wp, hp, op.mult)
                nc.vector.tensor_scalar(ir[:], ir[:], EPS_IR, None, op.max)

                sab = pairp.tile([128, MF], f16, tag="sab", name="sab")
                av = areaa[:, tf].unsqueeze(1).broadcast_to(PMF)
                abv = areabe[:].rearrange("p (m f) -> p m f", m=M)
                nc.gpsimd.tensor_tensor(
                    sab[:].rearrange("p (m f) -> p m f", m=M), av, abv, op.add)

                nc.scalar.activation(ir[:], ir[:], act.Ln)
                nc.scalar.activation(sab[:], sab[:], act.Ln)
                lq = ir
                nc.vector.tensor_tensor(lq[:], lq[:], sab[:], op.subtract)

                # tree max over m (5 halvings); (m,f) layout makes each step a
                # flat contiguous halved slice -> stays in 2x packed mode
                lqv = lq[:].rearrange("p (m f) -> p m f", m=M)
                tr16 = treep.tile([128, 16 * F], f16, tag="tr16", name="tr16")
                nc.vector.tensor_tensor(tr16[:], lq[:, 0:16 * F],
                                        lq[:, 16 * F:32 * F], op.max)
                tr8 = treep.tile([128, 8 * F], f16, tag="tr8", name="tr8")
                nc.vector.tensor_tensor(tr8[:], tr16[:, 0:8 * F],
                                        tr16[:, 8 * F:16 * F], op.max)
                tr4 = treep.tile([128, 4 * F], f16, tag="tr4", name="tr4")
                nc.vector.tensor_tensor(tr4[:], tr8[:, 0:4 * F],
                                        tr8[:, 4 * F:8 * F], op.max)
                tr2 = treep.tile([128, 2 * F], f16, tag="tr2", name="tr2")
                nc.vector.tensor_tensor(tr2[:], tr4[:, 0:2 * F],
                                        tr4[:, 2 * F:4 * F], op.max)
                mx = smallp.tile([128, F], f16, tag="mx", name="mx")
                nc.vector.tensor_tensor(mx[:], tr2[:, 0:F], tr2[:, F:2 * F], op.max)

                posf = smallp.tile([128, F], f16, tag="posf", name="posf")
                nc.vector.tensor_scalar(posf[:], mx[:], LN_POS, None, op.is_ge)
                negf = smallp.tile([128, F], f16, tag="negf", name="negf")
                nc.vector.tensor_scalar(negf[:], mx[:], LN_NEG, None, op.is_lt)
                # mxp = mx*posf + (1-posf)*1e4 : equals mx exactly on pos rows
                u1 = smallp.tile([128, F], f16, tag="u1", name="u1")
                nc.vector.tensor_scalar(u1[:], posf[:], -1e4, 1e4, op.mult, op.add)
                mxp = smallp.tile([128, F], f16, tag="mxp", name="mxp")
                nc.vector.tensor_tensor(mxp[:], mx[:], posf[:], op.mult)
                nc.vector.tensor_tensor(mxp[:], mxp[:], u1[:], op.add)

                # d = lq - mxp (GPSIMD, contiguous (m,f)); then one scalar-engine
                # activation builds the exact 0/1 mask in (f,m) order:
                # relu(1 + 1e4*d) = 1 iff d == 0 (nonzero |d| >= fp16 ulp ~5e-4)
                dq = pairp.tile([128, MF], f16, tag="dq", name="dq")
                dqv = dq[:].rearrange("p (m f) -> p m f", m=M)
                nc.gpsimd.tensor_tensor(
                    dqv, lqv, mxp[:].unsqueeze(1).broadcast_to(PMF), op.subtract)
                amask = pairp.tile([128, MF], f16, tag="amask", name="amask")
                nc.scalar.activation(
                    amask[:].rearrange("p (f m) -> p f m", m=M),
                    dq[:].rearrange("p (m f) -> p f m", f=F),
                    act.Relu, bias=1.0, scale=1e4)

                # ---- CE ----  (cls laid out c-outer, f-inner per tile)
                ctt = clsp.tile([128, CP * F], f16, tag="ctt", name="ctt")
                nc.sync.dma_start(ctt[:], ct_d[:, t * CP * F:(t + 1) * CP * F])
                ctv = ctt[:].rearrange("p (c f) -> p c f", f=F)
                ett = clsp.tile([128, CP * F], f16, tag="ett", name="ett")
                nc.scalar.activation(ett[:], ctt[:], act.Exp)
                # class-sum as flat pairwise tree on GPSIMD
                e12 = treep.tile([128, F * 12], f16, tag="e12", name="e12")
                nc.gpsimd.tensor_tensor(e12[:], ett[:, 0:12 * F],
                                        ett[:, 12 * F:24 * F], op.add)
                e6 = treep.tile([128, F * 6], f16, tag="e6", name="e6")
                nc.gpsimd.tensor_tensor(e6[:], e12[:, 0:6 * F],
                                        e12[:, 6 * F:12 * F], op.add)
                e3 = treep.tile([128, F * 3], f16, tag="e3", name="e3")
                nc.gpsimd.tensor_tensor(e3[:], e6[:, 0:3 * F],
                                        e6[:, 3 * F:6 * F], op.add)
                se = smallp.tile([128, F], f32, tag="se", name="se")
                nc.gpsimd.tensor_tensor(se[:], e3[:, F:2 * F], e3[:, 2 * F:3 * F],
                                        op.add)
                nc.gpsimd.tensor_tensor(se[:], se[:], e3[:, 0:F], op.add)
                lse = smallp.tile([128, F], f16, tag="lse", name="lse")
                nc.scalar.activation(lse[:], se[:], act.Ln)
                cls0 = ctv[:, 0, :]  # contiguous view, no copy needed

                # ---- counts + CE sums into staging ----
                scr = smallp.tile([128, F], f16, tag="scr", name="scr")
                nc.vector.tensor_scalar(
                    scr[:], posf[:], 1.0, 0.0, op.mult, op.add,
                    accum_out=staging[:, t * NQ + 0: t * NQ + 1])
                scr1 = smallp.tile([128, F], f16, tag="scr1", name="scr1")
                nc.vector.tensor_scalar(
                    scr1[:], negf[:], 1.0, 0.0, op.mult, op.add,
                    accum_out=staging[:, t * NQ + 1: t * NQ + 2])
                scr2 = smallp.tile([128, F], f16, tag="scr2", name="scr2")
                nc.vector.scalar_tensor_tensor(
                    scr2[:], posf[:], 1.0, lse[:], op.mult, op.mult,
                    accum_out=staging[:, t * NQ + 2: t * NQ + 3])
                scr3 = smallp.tile([128, F], f16, tag="scr3", name="scr3")
                nc.vector.scalar_tensor_tensor(
                    scr3[:], negf[:], 1.0, lse[:], op.mult, op.mult,
                    accum_out=staging[:, t * NQ + 3: t * NQ + 4])
                scr4 = smallp.tile([128, F], f16, tag="scr4", name="scr4")
                nc.vector.scalar_tensor_tensor(
                    scr4[:], negf[:], 1.0, cls0, op.mult, op.mult,
                    accum_out=staging[:, t * NQ + 4: t * NQ + 5])

                # ---- PE: transpose amask chunks + payload matmuls ----
                g_ps = psgp.tile([128, NCHUNK * 32], f32, tag="g_ps", name="g_ps")
                done = 0
                while done < NCHUNK:
                    gn = min(GRP, NCHUNK - done)
                    ps_t = pstp.tile([128, GRP * 128], f16, tag="ps_t", name="ps_t")
                    for c in range(gn):
                        ch = done + c
                        inv = amask[:, ch * 128:(ch + 1) * 128]
                        nc.tensor.transpose(
                            ps_t[:, c * 128:(c + 1) * 128], inv, ident[:])
                    amt = masktp.tile([128, GRP * 128], f16, tag="amt", name="amt")
                    nc.scalar.activation(amt[:, 0:gn * 128], ps_t[:, 0:gn * 128],
                                         act.Copy)
                    for c in range(gn):
                        ch = done + c
                        nc.tensor.matmul(
                            g_ps[:, ch * 32:(ch + 1) * 32],
                            amt[:, c * 128:(c + 1) * 128], pblk[:],
                            start=True, stop=True)
                    done += gn

                gsb = regsp.tile([128, NCHUNK * 32], f16, tag="gsb", name="gsb")
                nc.scalar.activation(gsb[:], g_ps[:], act.Copy)
                gv = gsb[:].rearrange("p (f k) -> p f k", k=8)

                # ---- gmat: G[m,c] += amask_f^T @ cls_f ----
                for f in range(F):
                    nc.tensor.matmul(
                        gmat_ps[:, :], amask[:, f * M:(f + 1) * M], ctv[:, :, f],
                        start=(t == 0 and f == 0),
                        stop=(t == NT - 1 and f == F - 1))

                # ---- reg loss ----
                cntc = smallp.tile([128, F], f16, tag="cntc", name="cntc")
                nc.vector.tensor_scalar(cntc[:], gv[:, :, 4], 1.0, None, op.max)
                rc = smallp.tile([128, F], f32, tag="rc", name="rc")
                nc.vector.reciprocal(rc[:], cntc[:])

                def rtile(tag):
                    return regsp.tile([128, F * 4], f16, tag=tag, name=tag)

                t1 = rtile("t1")
                t1v = t1[:].rearrange("p (f k) -> p f k", k=4)
                nc.vector.tensor_tensor(
                    t1v, gv[:, :, 0:4],
                    rc[:].unsqueeze(2).broadcast_to([128, F, 4]), op.mult)
                nc.vector.tensor_tensor(t1v, t1v, sview[:, tf, :], op.mult)
                dd = rtile("dd")
                ddv = dd[:].rearrange("p (f k) -> p f k", k=4)
                nc.vector.tensor_tensor(ddv, rview[:, tf, :], bview[:, tf, :],
                                        op.subtract)
                nc.vector.tensor_tensor(dd[:], dd[:], t1[:], op.subtract)
                ad = rtile("ad")
                nc.scalar.activation(ad[:], dd[:], act.Abs)
                cc = rtile("cc")
                ccv = cc[:].rearrange("p (f k) -> p f k", k=4)
                nc.vector.scalar_tensor_tensor(
                    ccv, ad[:].rearrange("p (f k) -> p f k", k=4), 1.0,
                    posf[:].unsqueeze(2).broadcast_to([128, F, 4]),
                    op.min, op.mult)
                t3 = rtile("t3")
                nc.vector.scalar_tensor_tensor(t3[:], cc[:], -0.5, ad[:],
                                               op.mult, op.add)
                sl1 = rtile("sl1")
                nc.vector.scalar_tensor_tensor(
                    sl1[:], cc[:], 1.0, t3[:], op.mult, op.mult,
                    accum_out=staging[:, t * NQ + 5: t * NQ + 6])

            gout = constp.tile([M, CP], f32, tag="gout", name="gout")
            nc.scalar.activation(gout[:], gmat_ps[:], act.Copy)
            nc.sync.dma_start(gmat_d[:], gout[:])
            nc.sync.dma_start(stage_d[:], staging[:])

    nc.compile()
    return nc


def _combine(stages, gmats, labels_list):
    cls_losses, reg_losses, n_pos_list = [], [], []
    ln21 = np.log(21.0)
    for st, gm, labs in zip(stages, gmats, labels_list):
        s = st.reshape(128, NT, NQ).sum(axis=(0, 1), dtype=np.float64)
        n_pos = s[0]
        n_neg = s[1] - NPAD_EXTRA
        plse = s[2]
        nlse = s[3] - NPAD_EXTRA * ln21
        ncls0 = s[4]
        psl1 = s[5]
        pcls = gm[np.arange(M), labs].sum(dtype=np.float64)
        cl = (plse - pcls) / max(n_pos, 1) + (nlse - ncls0) / max(n_neg, 1)
        rl = psl1 / max(4 * n_pos, 1)
        cls_losses.append(cl)
        reg_losses.append(rl)
        n_pos_list.append(n_pos)
    total_pos = int(round(sum(n_pos_list)))
    cls_final = np.float32(np.mean(np.array(cls_losses)))
    reg_final = np.float32(np.sum(np.array(reg_losses)) / max(total_pos, 1))
    total = np.float32(cls_final + reg_final)
    return total, cls_final, reg_final, np.int32(total_pos)


def kernel(cls_output, reg_output, anchors, target_boxes, target_labels):
    global LAST_RESULTS
    import os
    from concourse.bass_utils import run_bass_kernel_spmd

    if "nc" not in _CACHE:
        _CACHE["nc"] = _build_nc()
    nc = _CACHE["nc"]

    acon_t, s_t, b_t, ident = _host_prep_shared(anchors)
    in_maps = []
    for i in range(B):
        ct, rt, bxy0, bxy1, areab, pblk = _host_prep_image(
            cls_output[i], reg_output[i], target_boxes[i], target_labels[i])
        in_maps.append(dict(ct=ct, rt=rt, acon=acon_t, sv=s_t, bv=b_t,
                            bxy0=bxy0, bxy1=bxy1, areab=areab, pblk=pblk,
                            ident=ident))

    trace = os.environ.get("DETLOSS_TRACE", "0") == "1"
    res = run_bass_kernel_spmd(nc, in_maps, core_ids=list(range(B)), trace=trace)
    LAST_RESULTS = res
    stages = [r["stage"] for r in res.results]
    gmats = [r["gmat"] for r in res.results]
    labels_f = [np.asarray(target_labels[i]).astype(np.int64) for i in range(B)]
    return _combine(stages, gmats, labels_f)


if __name__ == "__main__":
    data = np.load("/root/problem/ref_inputs.npz")
    out = kernel(data["cls_output"], data["reg_output"], data["anchors"],
                 data["target_boxes"], data["target_labels"])
    print("kernel out:", [float(o) for o in out])
